# revision 36
# baseline (speedup 1.0000x reference)
# Bass/Trainium2 kernel for nn_AA2_Module_75359496175785 (sparse_attention).
#
# Math (per batch item b; x: (C,N) with C=128, N=H*W=16384):
#   q  = Wq x + bq;  k_g = Wk_g pool(x) + bk_g   (pooling commutes with 1x1 conv)
#   e_g = q^T k_g;   a_g = softmax(alpha_g e_g, axis=keys)
#   out = gamma0 k_0 a_0^T + x + gamma1 k_1 a_1^T
#
# v2 structure (bf16 I/O + elementwise rebalance):
#  * x is cast to bf16 HOST-side; input DMA is 4 MiB instead of 8 (phase 0
#    ~11us instead of ~22).  Output is written bf16 and upcast host-side.
#  * Phase 0 streams 8 pieces of 2048 cols; pooling = 16 identity matmuls
#    per piece accumulating row sums in PSUM (PE is otherwise idle, stays
#    warm for phase 1 with no separate burn; sums are exact f32) + a tiny
#    128-free DVE reduce per piece.  Last piece split 2x1024 for the tail.
#  * Phase 1 per 1024-col pair (gpsimd cannot touch PSUM, so the psum->sbuf
#    copies live on ACT+DVE):
#      PE : energy 2MM | sums 2MM | out 2MM + idb x-add MM (half 0)
#      ACT: exp (1024)            | psum->sbuf copy half 0
#      DVE: recip (1024)          | stt psum+x half 1
#      GPS: attn-mul (1024, one op)
import ml_dtypes  # noqa: F401
import numpy as np

B, C, H, W = 8, 128, 128, 128
N = H * W
PP = 8
NKEYS = 64
PIECE = 2048      # phase-0 dma piece = 16 rows of H = one pool-block row
NPIECE = N // PIECE
GRP = 512
PAIR = 2 * GRP    # phase-1 iteration width
NPAIR = N // PAIR
SKEW = 3          # phase-1 pipeline depth between energy(g) and out(g-SKEW)

_CACHE = {}


def _build_nc():
    import concourse.bass as bass  # noqa: F401
    from concourse import bacc, mybir
    import concourse.tile as tile

    f32 = mybir.dt.float32
    bf16 = mybir.dt.bfloat16
    AF = mybir.ActivationFunctionType
    MUL = mybir.AluOpType.mult
    ADD = mybir.AluOpType.add

    nc = bacc.Bacc(None, target_bir_lowering=False)

    x_d = nc.dram_tensor("x", [C, N], bf16, kind="ExternalInput")
    # bf16 weights: [WmT0 | WmT1 | WksT0 | WksT1 | idb | ones_bd]
    wb_d = nc.dram_tensor("wb", [C, 6 * C], bf16, kind="ExternalInput")
    # f32 weights: [bks0 bks1 bq gvec]
    wf_d = nc.dram_tensor("wf", [C, 7], f32, kind="ExternalInput")
    out_d = nc.dram_tensor("out", [C, N], bf16, kind="ExternalOutput")

    with tile.TileContext(nc) as tc:
        with (
            tc.tile_pool(name="const", bufs=1) as const,
            tc.tile_pool(name="big", bufs=1) as big,
            tc.tile_pool(name="tree", bufs=2) as tree,
            tc.tile_pool(name="expp", bufs=5) as expp,
            tc.tile_pool(name="rp", bufs=4) as rp,
            tc.tile_pool(name="attnp", bufs=5) as attnp,
            tc.tile_pool(name="outp", bufs=5) as outp,
        ):
            wb = const.tile([C, 6 * C], bf16)
            wf = const.tile([C, 7], f32)
            wmT0 = wb[:, 0:C]
            wmT1 = wb[:, C:2 * C]
            wksT0 = wb[:, 2 * C:3 * C]
            wksT1 = wb[:, 3 * C:4 * C]
            idb = wb[:, 4 * C:5 * C]
            ones_bd = wb[:, 5 * C:6 * C]
            bks0 = wf[:, 0:1]
            bks1 = wf[:, 1:2]
            gvec = wf[:, 3:4]
            wvb = wf[:, 4:6]   # Wks_g^T bq per branch (for the fast ebias)
            cvec = wf[:, 6:7]  # bq . bks_g per key's branch

            x_bf = big.tile([C, N], bf16)
            xp = big.tile([C, NKEYS], f32)
            xp_bf = big.tile([C, NKEYS], bf16)
            m_bf = big.tile([C, C], bf16)
            k_bf = big.tile([C, C], bf16)
            kT = big.tile([C, C], bf16)
            ebias = big.tile([C, 1], f32)
            tiny = big.tile([C, 1], f32)

            # ---- phase 0: stream x (bf16); pool on the PE ----
            # Row sums of each 16-row pool block accumulate in PSUM via 16
            # identity matmuls per piece (exact f32 adds, and the steady MM
            # stream holds the PE clock gate at 2.4 GHz through the mid with
            # no separate warm burn).  DVE only does a tiny 128-free reduce
            # over w=16 per piece.
            ph_pp = tc.tile_pool(name="ps_pp", bufs=1, space="PSUM")
            ps_pp = ph_pp.__enter__()
            pp_ps = ps_pp.tile([C, NPIECE * H // 2], f32, tag="poolps")
            scr_ps = ps_pp.tile([C, C], f32, tag="scr")

            # x head pieces first, then weights - input stream is critical
            for p in range(2):
                psl = bass.ts(p, PIECE)
                nc.sync.dma_start(x_bf[:, psl], x_d[:, psl])
            nc.sync.dma_start(wb[:], wb_d[:])
            nc.sync.dma_start(wf[:], wf_d[:])
            # preload the exp table set + prime small constants
            nc.scalar.activation(tiny[:], wf[:, 0:1], AF.Exp)
            wvb_bf = big.tile([C, 2], bf16)
            nc.vector.tensor_copy(wvb_bf[:], wvb)
            for p in range(NPIECE):
                base = p * PIECE
                if p == NPIECE - 1:
                    # split the final piece so its pooling can start on the
                    # first 8 rows while the last 8 stream in
                    nc.sync.dma_start(
                        x_bf[:, bass.ds(base, PIECE // 2)],
                        x_d[:, bass.ds(base, PIECE // 2)],
                    )
                    nc.sync.dma_start(
                        x_bf[:, bass.ds(base + PIECE // 2, PIECE // 2)],
                        x_d[:, bass.ds(base + PIECE // 2, PIECE // 2)],
                    )
                elif p >= 2:
                    nc.sync.dma_start(
                        x_bf[:, bass.ts(p, PIECE)], x_d[:, bass.ts(p, PIECE)]
                    )
                if p % 2 == 1:
                    # odd pieces (incl. the split tail): PE row-sums via 16
                    # accumulating identity matmuls into PSUM + tiny reduce
                    pps = pp_ps[:, (p // 2) * H:(p // 2 + 1) * H]
                    for r in range(16):
                        nc.tensor.matmul(
                            pps, idb, x_bf[:, bass.ds(base + r * H, H)],
                            start=(r == 0), stop=(r == 15),
                        )
                    ppv = pps.rearrange("p (pj w) -> p pj w", pj=PP, w=16)
                    nc.vector.tensor_reduce(
                        xp[:, p * PP:(p + 1) * PP], ppv,
                        axis=mybir.AxisListType.X, op=ADD,
                    )
                else:
                    # even pieces: DVE bf16 2x tree (rows 16->8->4) + reduce
                    t1 = tree.tile([C, PIECE // 2], bf16, name="t1")
                    t2 = tree.tile([C, PIECE // 4], bf16, name="t2")
                    nc.vector.tensor_tensor(
                        out=t1[:], in0=x_bf[:, bass.ds(base, 1024)],
                        in1=x_bf[:, bass.ds(base + 1024, 1024)], op=ADD,
                    )
                    nc.vector.tensor_tensor(
                        out=t2[:], in0=t1[:, 0:512], in1=t1[:, 512:1024], op=ADD,
                    )
                    t2v = t2[:].rearrange("p (h pj w) -> p pj h w", h=4, pj=PP, w=16)
                    nc.vector.tensor_reduce(
                        xp[:, p * PP:(p + 1) * PP], t2v,
                        axis=mybir.AxisListType.XY, op=ADD,
                    )
                if p == 4:
                    # contiguous ~4.8us matmul burn: the parity-split pool
                    # matmuls alone are too sparse to trip the HAM activity
                    # window, leaving the whole mid at 1.2 GHz
                    for _ in range(45):
                        nc.tensor.matmul(
                            scr_ps[:], ones_bd, x_bf[:, bass.ds(base, C)],
                            start=True, stop=True,
                        )

            ph_pp.__exit__(None, None, None)

            # ---- mid: M (energy weights), keys, kT, ebias ----
            # Ordering: energy(0)+exp(0) are emitted BEFORE the kk/kT chain
            # so the first pair's PE+ACT work never queues behind the
            # transpose (which waits on the k_bf activations).  ps_e opens
            # while ps0 (4 banks) is still live: 4+4 = 8 banks exactly.
            ph_e = tc.tile_pool(name="ps_e", bufs=2, space="PSUM")
            ps_e = ph_e.__enter__()
            ph0 = tc.tile_pool(name="ps0", bufs=1, space="PSUM")
            ps0 = ph0.__enter__()
            m_ps = ps0.tile([C, C], f32, tag="mps")
            kk_ps = ps0.tile([C, C], f32, tag="kkps")
            eb_ps = ps0.tile([C, 1], f32, tag="ebps")
            kT_ps = ps0.tile([C, C], bf16, tag="ktps")

            nc.vector.tensor_copy(xp_bf[:], xp[:])
            nc.tensor.matmul(m_ps[:, 0:NKEYS], wmT0, xp_bf[:], start=True, stop=True)
            nc.tensor.matmul(m_ps[:, NKEYS:], wmT1, xp_bf[:], start=True, stop=True)
            # fast ebias: ebias[key] = bq . k_key = (Wks^T bq) . xp_key + bq.bks
            # - straight from pooled x, off the k_bf critical path
            nc.tensor.matmul(
                eb_ps[0:NKEYS, :], xp_bf[:], wvb_bf[:, 0:1], start=True, stop=True
            )
            nc.tensor.matmul(
                eb_ps[NKEYS:, :], xp_bf[:], wvb_bf[:, 1:2], start=True, stop=True
            )
            # NOTE: the M bias (Wq^T bks) is a per-column constant within each
            # branch's key group, so softmax cancels it - no bias needed here.
            nc.scalar.copy(m_bf[:], m_ps[:])
            nc.vector.tensor_scalar_add(out=ebias[:], in0=eb_ps[:], scalar1=cvec)

            exps = [None] * NPAIR
            attns = [None] * NPAIR

            def stage_a(ga):
                a0 = bass.ds(ga * PAIR, GRP)
                a1 = bass.ds(ga * PAIR + GRP, GRP)
                e_ps = ps_e.tile([C, PAIR], f32, tag="eps")
                nc.tensor.matmul(
                    e_ps[:, 0:GRP], m_bf[:], x_bf[:, a0], start=True, stop=True
                )
                nc.tensor.matmul(
                    e_ps[:, GRP:], m_bf[:], x_bf[:, a1], start=True, stop=True
                )
                exps[ga] = expp.tile([C, PAIR], bf16, name="exp_sb")
                nc.scalar.activation(
                    exps[ga][:], e_ps[:], AF.Exp, bias=ebias[:, 0:1], scale=1.0
                )

            stage_a(0)

            # k/kT chain - only needed by out(0) at iteration SKEW, so it
            # runs in the PE/ACT shadow of the first exp
            nc.tensor.matmul(kk_ps[:, 0:NKEYS], wksT0, xp_bf[:], start=True, stop=True)
            nc.tensor.matmul(kk_ps[:, NKEYS:], wksT1, xp_bf[:], start=True, stop=True)
            nc.scalar.activation(
                k_bf[:, 0:NKEYS], kk_ps[:, 0:NKEYS], AF.Identity, bias=bks0, scale=1.0
            )
            nc.scalar.activation(
                k_bf[:, NKEYS:], kk_ps[:, NKEYS:], AF.Identity, bias=bks1, scale=1.0
            )
            nc.tensor.transpose(kT_ps[:], k_bf[:], idb)
            nc.scalar.activation(kT[:], kT_ps[:], AF.Copy, scale=gvec)
            ph0.__exit__(None, None, None)

            # ---- phase 1: deep-skewed pipeline over 1024-column pairs ----
            ph_s = tc.tile_pool(name="ps_s", bufs=1, space="PSUM")
            ps_s = ph_s.__enter__()
            ph_u = tc.tile_pool(name="ps_u", bufs=1, space="PSUM")
            ps_u = ph_u.__enter__()

            last_o = [None]
            for g in range(1, NPAIR + SKEW):
                ga, gb, gc = g, g - 1, g - SKEW
                # oldest stage first so no engine queue ever waits behind a
                # younger, not-yet-ready instruction (head-of-line blocking)
                if 0 <= gc < NPAIR:
                    c0 = bass.ds(gc * PAIR, GRP)
                    c1 = bass.ds(gc * PAIR + GRP, GRP)
                    u0 = ps_u.tile([C, GRP], f32, tag="uh0")
                    u1 = ps_u.tile([C, GRP], f32, tag="uh1")
                    nc.tensor.matmul(
                        u0[:], kT[:], attns[gc][:, 0:GRP], start=True, stop=False
                    )
                    nc.tensor.matmul(
                        u1[:], kT[:], attns[gc][:, GRP:], start=True, stop=True
                    )
                    nc.tensor.matmul(
                        u0[:], idb, x_bf[:, c0], start=False, stop=True
                    )
                    o_sb = outp.tile([C, PAIR], bf16)
                    last_o[0] = o_sb
                    nc.scalar.activation(o_sb[:, 0:GRP], u0[:], AF.Copy)
                    nc.vector.scalar_tensor_tensor(
                        out=o_sb[:, GRP:],
                        in0=u1[:],
                        scalar=1.0,
                        in1=x_bf[:, c1],
                        op0=MUL,
                        op1=ADD,
                    )
                    nc.sync.dma_start(out_d[:, bass.ds(gc * PAIR, PAIR)], o_sb[:])
                if 0 <= gb < NPAIR:
                    s_ps = ps_s.tile([C, PAIR], f32, tag="sps")
                    nc.tensor.matmul(
                        s_ps[:, 0:GRP], ones_bd, exps[gb][:, 0:GRP],
                        start=True, stop=True,
                    )
                    nc.tensor.matmul(
                        s_ps[:, GRP:], ones_bd, exps[gb][:, GRP:],
                        start=True, stop=True,
                    )
                    r_sb = rp.tile([C, PAIR], f32)
                    nc.vector.reciprocal_approx_fast(
                        out=r_sb[:, 0:GRP], in_=s_ps[:, 0:GRP]
                    )
                    nc.vector.reciprocal_approx_fast(
                        out=r_sb[:, GRP:], in_=s_ps[:, GRP:]
                    )
                    # attn-mul as a single gpsimd op (amortizes the Q7 launch)
                    at = attnp.tile([C, PAIR], bf16, name="attn")
                    nc.gpsimd.tensor_mul(at[:], exps[gb][:], r_sb[:])
                    attns[gb] = at
                if ga < NPAIR:
                    stage_a(ga)
                else:
                    # drain keepalive: enough matmul activity per drain
                    # iteration that the PE clock gate never re-throttles
                    # (cold MMs double every remaining chain latency)
                    # rhs = this iteration's output tile: ties each filler
                    # set to the drain's progress so the matmul activity is
                    # spread across the drain instead of bunching at its start
                    f_ps = ps_s.tile([C, PAIR], f32, tag="sps")
                    for fh in range(2):
                        for _ in range(4):
                            nc.tensor.matmul(
                                f_ps[:, fh * GRP:(fh + 1) * GRP], ones_bd,
                                last_o[0][:, fh * GRP:(fh + 1) * GRP],
                                start=True, stop=True,
                            )
            ph_u.__exit__(None, None, None)
            ph_s.__exit__(None, None, None)
            ph_e.__exit__(None, None, None)

    nc.compile()
    return nc


def _get_nc():
    if "nc" not in _CACHE:
        _CACHE["nc"] = _build_nc()
    return _CACHE["nc"]


def _make_in_maps(x, Wq, bq, Wk, bk, Wk1, bk1, gamma, gamma1, aphal, aphal1):
    a0 = float(np.asarray(aphal).reshape(-1)[0])
    a1 = float(np.asarray(aphal1).reshape(-1)[0])
    g0 = float(np.asarray(gamma).reshape(-1)[0])
    g1 = float(np.asarray(gamma1).reshape(-1)[0])

    f = np.float32
    Wq = np.asarray(Wq, f)
    Wks0 = np.asarray(Wk, f) * (a0 / 256.0)
    Wks1 = np.asarray(Wk1, f) * (a1 / 256.0)
    bks0 = np.asarray(bk, f).reshape(C) * a0
    bks1 = np.asarray(bk1, f).reshape(C) * a1
    wmT0 = Wks0.T @ Wq           # stationary for M = (Wq^T Wks) pool(x)
    wmT1 = Wks1.T @ Wq
    eye = np.eye(C, dtype=f)
    ones_bd = np.kron(np.eye(2, dtype=f), np.ones((NKEYS, NKEYS), f))
    wb = np.concatenate(
        [wmT0, wmT1, Wks0.T, Wks1.T, eye, ones_bd], axis=1
    ).astype("bfloat16")
    gvec = np.concatenate(
        [np.full((NKEYS, 1), g0 / a0, f), np.full((NKEYS, 1), g1 / a1, f)]
    )
    bqv = np.asarray(bq, f).reshape(C)
    wvb0 = Wks0.T @ bqv
    wvb1 = Wks1.T @ bqv
    cvec = np.concatenate(
        [np.full((NKEYS, 1), float(bqv @ bks0), f),
         np.full((NKEYS, 1), float(bqv @ bks1), f)]
    )
    wf = np.concatenate(
        [
            bks0.reshape(C, 1), bks1.reshape(C, 1),
            bqv.reshape(C, 1),
            gvec,
            wvb0.reshape(C, 1), wvb1.reshape(C, 1),
            cvec,
        ],
        axis=1,
    ).astype(f)
    wb = np.ascontiguousarray(wb)
    wf = np.ascontiguousarray(wf)
    in_maps = []
    for b in range(B):
        xb = np.ascontiguousarray(
            np.asarray(x)[b].reshape(C, N).astype("bfloat16")
        )
        in_maps.append({
            "x": xb,
            "wb": wb,
            "wf": wf,
        })
    return in_maps


def kernel(x, Wq, bq, Wk, bk, Wk1, bk1, gamma, gamma1, aphal, aphal1, **_):
    from concourse.bass_utils import run_bass_kernel_spmd

    nc = _get_nc()
    in_maps = _make_in_maps(
        np.asarray(x), np.asarray(Wq), np.asarray(bq), np.asarray(Wk),
        np.asarray(bk), np.asarray(Wk1), np.asarray(bk1), np.asarray(gamma),
        np.asarray(gamma1), np.asarray(aphal), np.asarray(aphal1),
    )
    res = None
    last_exc = None
    for _attempt in range(3):
        try:
            res = run_bass_kernel_spmd(nc, in_maps, core_ids=list(range(B)))
            break
        except Exception as e:  # transient NRT_EXEC_UNIT_UNRECOVERABLE faults
            last_exc = e
            import time as _time
            _time.sleep(2.0)
    if res is None:
        raise last_exc
    out = np.stack([
        res.results[b]["out"].astype(np.float32).reshape(C, H, W)
        for b in range(B)
    ])
    return out


# revision 38
# speedup vs baseline: 1.0125x; 1.0125x over previous
# Bass/Trainium2 kernel for nn_AA2_Module_75359496175785 (sparse_attention).
#
# Math (per batch item b; x: (C,N) with C=128, N=H*W=16384):
#   q  = Wq x + bq;  k_g = Wk_g pool(x) + bk_g   (pooling commutes with 1x1 conv)
#   e_g = q^T k_g;   a_g = softmax(alpha_g e_g, axis=keys)
#   out = gamma0 k_0 a_0^T + x + gamma1 k_1 a_1^T
#
# v2 structure (bf16 I/O + elementwise rebalance):
#  * x is cast to bf16 HOST-side; input DMA is 4 MiB instead of 8 (phase 0
#    ~11us instead of ~22).  Output is written bf16 and upcast host-side.
#  * Phase 0 streams 8 pieces of 2048 cols; pooling = 16 identity matmuls
#    per piece accumulating row sums in PSUM (PE is otherwise idle, stays
#    warm for phase 1 with no separate burn; sums are exact f32) + a tiny
#    128-free DVE reduce per piece.  Last piece split 2x1024 for the tail.
#  * Phase 1 per 1024-col pair (gpsimd cannot touch PSUM, so the psum->sbuf
#    copies live on ACT+DVE):
#      PE : energy 2MM | sums 2MM | out 2MM + idb x-add MM (half 0)
#      ACT: exp (1024)            | psum->sbuf copy half 0
#      DVE: recip (1024)          | stt psum+x half 1
#      GPS: attn-mul (1024, one op)
import ml_dtypes  # noqa: F401
import numpy as np

B, C, H, W = 8, 128, 128, 128
N = H * W
PP = 8
NKEYS = 64
PIECE = 2048      # phase-0 dma piece = 16 rows of H = one pool-block row
NPIECE = N // PIECE
GRP = 512
PAIR = 2 * GRP    # phase-1 iteration width
NPAIR = N // PAIR
SKEW = 3          # phase-1 pipeline depth between energy(g) and out(g-SKEW)

_CACHE = {}


def _build_nc():
    import concourse.bass as bass  # noqa: F401
    from concourse import bacc, mybir
    import concourse.tile as tile

    f32 = mybir.dt.float32
    bf16 = mybir.dt.bfloat16
    AF = mybir.ActivationFunctionType
    MUL = mybir.AluOpType.mult
    ADD = mybir.AluOpType.add

    nc = bacc.Bacc(None, target_bir_lowering=False)

    x_d = nc.dram_tensor("x", [C, N], bf16, kind="ExternalInput")
    # bf16 weights: [WmT0 | WmT1 | WksT0 | WksT1 | idb | ones_bd]
    wb_d = nc.dram_tensor("wb", [C, 6 * C], bf16, kind="ExternalInput")
    # f32 weights: [bks0 bks1 bq gvec]
    wf_d = nc.dram_tensor("wf", [C, 7], f32, kind="ExternalInput")
    out_d = nc.dram_tensor("out", [C, N], bf16, kind="ExternalOutput")

    with tile.TileContext(nc) as tc:
        with (
            tc.tile_pool(name="const", bufs=1) as const,
            tc.tile_pool(name="big", bufs=1) as big,
            tc.tile_pool(name="tree", bufs=2) as tree,
            tc.tile_pool(name="expp", bufs=6) as expp,
            tc.tile_pool(name="rp", bufs=6) as rp,
            tc.tile_pool(name="attnp", bufs=6) as attnp,
            tc.tile_pool(name="outp", bufs=7) as outp,
        ):
            wb = const.tile([C, 6 * C], bf16)
            wf = const.tile([C, 7], f32)
            wmT0 = wb[:, 0:C]
            wmT1 = wb[:, C:2 * C]
            wksT0 = wb[:, 2 * C:3 * C]
            wksT1 = wb[:, 3 * C:4 * C]
            idb = wb[:, 4 * C:5 * C]
            ones_bd = wb[:, 5 * C:6 * C]
            bks0 = wf[:, 0:1]
            bks1 = wf[:, 1:2]
            gvec = wf[:, 3:4]
            wvb = wf[:, 4:6]   # Wks_g^T bq per branch (for the fast ebias)
            cvec = wf[:, 6:7]  # bq . bks_g per key's branch

            x_bf = big.tile([C, N], bf16)
            xp = big.tile([C, NKEYS], f32)
            xp_bf = big.tile([C, NKEYS], bf16)
            m_bf = big.tile([C, C], bf16)
            k_bf = big.tile([C, C], bf16)
            kT = big.tile([C, C], bf16)
            ebias = big.tile([C, 1], f32)
            tiny = big.tile([C, 1], f32)

            # ---- phase 0: stream x (bf16); pool on the PE ----
            # Row sums of each 16-row pool block accumulate in PSUM via 16
            # identity matmuls per piece (exact f32 adds, and the steady MM
            # stream holds the PE clock gate at 2.4 GHz through the mid with
            # no separate warm burn).  DVE only does a tiny 128-free reduce
            # over w=16 per piece.
            ph_pp = tc.tile_pool(name="ps_pp", bufs=1, space="PSUM")
            ps_pp = ph_pp.__enter__()
            pp_ps = ps_pp.tile([C, NPIECE * H // 2], f32, tag="poolps")
            scr_ps = ps_pp.tile([C, C], f32, tag="scr")

            # x head pieces first, then weights - input stream is critical
            for p in range(2):
                psl = bass.ts(p, PIECE)
                nc.sync.dma_start(x_bf[:, psl], x_d[:, psl])
            nc.sync.dma_start(wb[:], wb_d[:])
            nc.sync.dma_start(wf[:], wf_d[:])
            # preload the exp table set + prime small constants
            nc.scalar.activation(tiny[:], wf[:, 0:1], AF.Exp)
            wvb_bf = big.tile([C, 2], bf16)
            nc.vector.tensor_copy(wvb_bf[:], wvb)
            for p in range(NPIECE):
                base = p * PIECE
                if p == NPIECE - 1:
                    # split the final piece so its pooling can start on the
                    # first 8 rows while the last 8 stream in
                    nc.sync.dma_start(
                        x_bf[:, bass.ds(base, PIECE // 2)],
                        x_d[:, bass.ds(base, PIECE // 2)],
                    )
                    nc.sync.dma_start(
                        x_bf[:, bass.ds(base + PIECE // 2, PIECE // 2)],
                        x_d[:, bass.ds(base + PIECE // 2, PIECE // 2)],
                    )
                elif p >= 2:
                    nc.sync.dma_start(
                        x_bf[:, bass.ts(p, PIECE)], x_d[:, bass.ts(p, PIECE)]
                    )
                if p % 2 == 1:
                    # odd pieces (incl. the split tail): PE row-sums via 16
                    # accumulating identity matmuls into PSUM + tiny reduce
                    pps = pp_ps[:, (p // 2) * H:(p // 2 + 1) * H]
                    for r in range(16):
                        nc.tensor.matmul(
                            pps, idb, x_bf[:, bass.ds(base + r * H, H)],
                            start=(r == 0), stop=(r == 15),
                        )
                    ppv = pps.rearrange("p (pj w) -> p pj w", pj=PP, w=16)
                    nc.vector.tensor_reduce(
                        xp[:, p * PP:(p + 1) * PP], ppv,
                        axis=mybir.AxisListType.X, op=ADD,
                    )
                else:
                    # even pieces: DVE bf16 2x tree (rows 16->8->4) + reduce
                    t1 = tree.tile([C, PIECE // 2], bf16, name="t1")
                    t2 = tree.tile([C, PIECE // 4], bf16, name="t2")
                    nc.vector.tensor_tensor(
                        out=t1[:], in0=x_bf[:, bass.ds(base, 1024)],
                        in1=x_bf[:, bass.ds(base + 1024, 1024)], op=ADD,
                    )
                    nc.vector.tensor_tensor(
                        out=t2[:], in0=t1[:, 0:512], in1=t1[:, 512:1024], op=ADD,
                    )
                    t2v = t2[:].rearrange("p (h pj w) -> p pj h w", h=4, pj=PP, w=16)
                    nc.vector.tensor_reduce(
                        xp[:, p * PP:(p + 1) * PP], t2v,
                        axis=mybir.AxisListType.XY, op=ADD,
                    )
                if p == 4:
                    # contiguous ~4.8us matmul burn: the parity-split pool
                    # matmuls alone are too sparse to trip the HAM activity
                    # window, leaving the whole mid at 1.2 GHz
                    for _ in range(45):
                        nc.tensor.matmul(
                            scr_ps[:], ones_bd, x_bf[:, bass.ds(base, C)],
                            start=True, stop=True,
                        )

            ph_pp.__exit__(None, None, None)

            # ---- mid: M (energy weights), keys, kT, ebias ----
            # Ordering: energy(0)+exp(0) are emitted BEFORE the kk/kT chain
            # so the first pair's PE+ACT work never queues behind the
            # transpose (which waits on the k_bf activations).  ps_e opens
            # while ps0 (4 banks) is still live: 4+4 = 8 banks exactly.
            ph_e = tc.tile_pool(name="ps_e", bufs=2, space="PSUM")
            ps_e = ph_e.__enter__()
            ph0 = tc.tile_pool(name="ps0", bufs=1, space="PSUM")
            ps0 = ph0.__enter__()
            m_ps = ps0.tile([C, C], f32, tag="mps")
            kk_ps = ps0.tile([C, C], f32, tag="kkps")
            eb_ps = ps0.tile([C, 1], f32, tag="ebps")
            kT_ps = ps0.tile([C, C], bf16, tag="ktps")

            nc.vector.tensor_copy(xp_bf[:], xp[:])
            nc.tensor.matmul(m_ps[:, 0:NKEYS], wmT0, xp_bf[:], start=True, stop=True)
            nc.tensor.matmul(m_ps[:, NKEYS:], wmT1, xp_bf[:], start=True, stop=True)
            # fast ebias: ebias[key] = bq . k_key = (Wks^T bq) . xp_key + bq.bks
            # - straight from pooled x, off the k_bf critical path
            nc.tensor.matmul(
                eb_ps[0:NKEYS, :], xp_bf[:], wvb_bf[:, 0:1], start=True, stop=True
            )
            nc.tensor.matmul(
                eb_ps[NKEYS:, :], xp_bf[:], wvb_bf[:, 1:2], start=True, stop=True
            )
            # NOTE: the M bias (Wq^T bks) is a per-column constant within each
            # branch's key group, so softmax cancels it - no bias needed here.
            nc.scalar.copy(m_bf[:], m_ps[:])
            nc.vector.tensor_scalar_add(out=ebias[:], in0=eb_ps[:], scalar1=cvec)

            exps = [None] * NPAIR
            attns = [None] * NPAIR

            def stage_a(ga):
                a0 = bass.ds(ga * PAIR, GRP)
                a1 = bass.ds(ga * PAIR + GRP, GRP)
                e_ps = ps_e.tile([C, PAIR], f32, tag="eps")
                nc.tensor.matmul(
                    e_ps[:, 0:GRP], m_bf[:], x_bf[:, a0], start=True, stop=True
                )
                nc.tensor.matmul(
                    e_ps[:, GRP:], m_bf[:], x_bf[:, a1], start=True, stop=True
                )
                exps[ga] = expp.tile([C, PAIR], bf16, name="exp_sb")
                nc.scalar.activation(
                    exps[ga][:], e_ps[:], AF.Exp, bias=ebias[:, 0:1], scale=1.0
                )

            stage_a(0)

            # k/kT chain - only needed by out(0) at iteration SKEW, so it
            # runs in the PE/ACT shadow of the first exp
            nc.tensor.matmul(kk_ps[:, 0:NKEYS], wksT0, xp_bf[:], start=True, stop=True)
            nc.tensor.matmul(kk_ps[:, NKEYS:], wksT1, xp_bf[:], start=True, stop=True)
            nc.scalar.activation(
                k_bf[:, 0:NKEYS], kk_ps[:, 0:NKEYS], AF.Identity, bias=bks0, scale=1.0
            )
            nc.scalar.activation(
                k_bf[:, NKEYS:], kk_ps[:, NKEYS:], AF.Identity, bias=bks1, scale=1.0
            )
            nc.tensor.transpose(kT_ps[:], k_bf[:], idb)
            nc.scalar.activation(kT[:], kT_ps[:], AF.Copy, scale=gvec)
            ph0.__exit__(None, None, None)

            # ---- phase 1: deep-skewed pipeline over 1024-column pairs ----
            ph_s = tc.tile_pool(name="ps_s", bufs=1, space="PSUM")
            ps_s = ph_s.__enter__()
            ph_u = tc.tile_pool(name="ps_u", bufs=1, space="PSUM")
            ps_u = ph_u.__enter__()

            for g in range(1, NPAIR + SKEW):
                ga, gb, gc = g, g - 1, g - SKEW
                # oldest stage first so no engine queue ever waits behind a
                # younger, not-yet-ready instruction (head-of-line blocking)
                if 0 <= gc < NPAIR:
                    c0 = bass.ds(gc * PAIR, GRP)
                    c1 = bass.ds(gc * PAIR + GRP, GRP)
                    u0 = ps_u.tile([C, GRP], f32, tag="uh0")
                    u1 = ps_u.tile([C, GRP], f32, tag="uh1")
                    nc.tensor.matmul(
                        u0[:], kT[:], attns[gc][:, 0:GRP], start=True, stop=False
                    )
                    nc.tensor.matmul(
                        u1[:], kT[:], attns[gc][:, GRP:], start=True, stop=True
                    )
                    nc.tensor.matmul(
                        u0[:], idb, x_bf[:, c0], start=False, stop=True
                    )
                    o_sb = outp.tile([C, PAIR], bf16)
                    nc.scalar.activation(o_sb[:, 0:GRP], u0[:], AF.Copy)
                    nc.vector.scalar_tensor_tensor(
                        out=o_sb[:, GRP:],
                        in0=u1[:],
                        scalar=1.0,
                        in1=x_bf[:, c1],
                        op0=MUL,
                        op1=ADD,
                    )
                    # two half DMAs: the ACT-copied half streams out while
                    # the DVE stt half is still being produced
                    nc.sync.dma_start(
                        out_d[:, bass.ds(gc * PAIR, GRP)], o_sb[:, 0:GRP]
                    )
                    nc.sync.dma_start(
                        out_d[:, bass.ds(gc * PAIR + GRP, GRP)], o_sb[:, GRP:]
                    )
                if 0 <= gb < NPAIR:
                    s_ps = ps_s.tile([C, PAIR], f32, tag="sps")
                    nc.tensor.matmul(
                        s_ps[:, 0:GRP], ones_bd, exps[gb][:, 0:GRP],
                        start=True, stop=True,
                    )
                    nc.tensor.matmul(
                        s_ps[:, GRP:], ones_bd, exps[gb][:, GRP:],
                        start=True, stop=True,
                    )
                    r_sb = rp.tile([C, PAIR], f32)
                    nc.vector.reciprocal_approx_fast(
                        out=r_sb[:, 0:GRP], in_=s_ps[:, 0:GRP]
                    )
                    nc.vector.reciprocal_approx_fast(
                        out=r_sb[:, GRP:], in_=s_ps[:, GRP:]
                    )
                    # attn-mul as a single gpsimd op (amortizes the Q7 launch)
                    at = attnp.tile([C, PAIR], bf16, name="attn")
                    nc.gpsimd.tensor_mul(at[:], exps[gb][:], r_sb[:])
                    attns[gb] = at
                if ga < NPAIR:
                    stage_a(ga)
                else:
                    # drain keepalive: enough matmul activity per drain
                    # iteration that the PE clock gate never re-throttles
                    # (cold MMs double every remaining chain latency)
                    f_ps = ps_s.tile([C, PAIR], f32, tag="sps")
                    for fh in range(2):
                        for _ in range(3):
                            nc.tensor.matmul(
                                f_ps[:, fh * GRP:(fh + 1) * GRP], ones_bd,
                                x_bf[:, bass.ds(fh * GRP, GRP)],
                                start=True, stop=True,
                            )
            ph_u.__exit__(None, None, None)
            ph_s.__exit__(None, None, None)
            ph_e.__exit__(None, None, None)

    nc.compile()
    return nc


def _get_nc():
    if "nc" not in _CACHE:
        _CACHE["nc"] = _build_nc()
    return _CACHE["nc"]


def _make_in_maps(x, Wq, bq, Wk, bk, Wk1, bk1, gamma, gamma1, aphal, aphal1):
    a0 = float(np.asarray(aphal).reshape(-1)[0])
    a1 = float(np.asarray(aphal1).reshape(-1)[0])
    g0 = float(np.asarray(gamma).reshape(-1)[0])
    g1 = float(np.asarray(gamma1).reshape(-1)[0])

    f = np.float32
    Wq = np.asarray(Wq, f)
    Wks0 = np.asarray(Wk, f) * (a0 / 256.0)
    Wks1 = np.asarray(Wk1, f) * (a1 / 256.0)
    bks0 = np.asarray(bk, f).reshape(C) * a0
    bks1 = np.asarray(bk1, f).reshape(C) * a1
    wmT0 = Wks0.T @ Wq           # stationary for M = (Wq^T Wks) pool(x)
    wmT1 = Wks1.T @ Wq
    eye = np.eye(C, dtype=f)
    ones_bd = np.kron(np.eye(2, dtype=f), np.ones((NKEYS, NKEYS), f))
    wb = np.concatenate(
        [wmT0, wmT1, Wks0.T, Wks1.T, eye, ones_bd], axis=1
    ).astype("bfloat16")
    gvec = np.concatenate(
        [np.full((NKEYS, 1), g0 / a0, f), np.full((NKEYS, 1), g1 / a1, f)]
    )
    bqv = np.asarray(bq, f).reshape(C)
    wvb0 = Wks0.T @ bqv
    wvb1 = Wks1.T @ bqv
    cvec = np.concatenate(
        [np.full((NKEYS, 1), float(bqv @ bks0), f),
         np.full((NKEYS, 1), float(bqv @ bks1), f)]
    )
    wf = np.concatenate(
        [
            bks0.reshape(C, 1), bks1.reshape(C, 1),
            bqv.reshape(C, 1),
            gvec,
            wvb0.reshape(C, 1), wvb1.reshape(C, 1),
            cvec,
        ],
        axis=1,
    ).astype(f)
    wb = np.ascontiguousarray(wb)
    wf = np.ascontiguousarray(wf)
    in_maps = []
    for b in range(B):
        xb = np.ascontiguousarray(
            np.asarray(x)[b].reshape(C, N).astype("bfloat16")
        )
        in_maps.append({
            "x": xb,
            "wb": wb,
            "wf": wf,
        })
    return in_maps


def kernel(x, Wq, bq, Wk, bk, Wk1, bk1, gamma, gamma1, aphal, aphal1, **_):
    from concourse.bass_utils import run_bass_kernel_spmd

    nc = _get_nc()
    in_maps = _make_in_maps(
        np.asarray(x), np.asarray(Wq), np.asarray(bq), np.asarray(Wk),
        np.asarray(bk), np.asarray(Wk1), np.asarray(bk1), np.asarray(gamma),
        np.asarray(gamma1), np.asarray(aphal), np.asarray(aphal1),
    )
    res = None
    last_exc = None
    for _attempt in range(3):
        try:
            res = run_bass_kernel_spmd(nc, in_maps, core_ids=list(range(B)))
            break
        except Exception as e:  # transient NRT_EXEC_UNIT_UNRECOVERABLE faults
            last_exc = e
            import time as _time
            _time.sleep(2.0)
    if res is None:
        raise last_exc
    out = np.stack([
        res.results[b]["out"].astype(np.float32).reshape(C, H, W)
        for b in range(B)
    ])
    return out


# revision 40
# speedup vs baseline: 1.0647x; 1.0516x over previous
# Bass/Trainium2 kernel for nn_AA2_Module_75359496175785 (sparse_attention).
#
# Math (per batch item b; x: (C,N) with C=128, N=H*W=16384):
#   q  = Wq x + bq;  k_g = Wk_g pool(x) + bk_g   (pooling commutes with 1x1 conv)
#   e_g = q^T k_g;   a_g = softmax(alpha_g e_g, axis=keys)
#   out = gamma0 k_0 a_0^T + x + gamma1 k_1 a_1^T
#
# v2 structure (bf16 I/O + elementwise rebalance):
#  * x is cast to bf16 HOST-side; input DMA is 4 MiB instead of 8 (phase 0
#    ~11us instead of ~22).  Output is written bf16 and upcast host-side.
#  * Phase 0 streams 8 pieces of 2048 cols; pooling = 16 identity matmuls
#    per piece accumulating row sums in PSUM (PE is otherwise idle, stays
#    warm for phase 1 with no separate burn; sums are exact f32) + a tiny
#    128-free DVE reduce per piece.  Last piece split 2x1024 for the tail.
#  * Phase 1 per 1024-col pair (gpsimd cannot touch PSUM, so the psum->sbuf
#    copies live on ACT+DVE):
#      PE : energy 2MM | sums 2MM | out 2MM + idb x-add MM (half 0)
#      ACT: exp (1024)            | psum->sbuf copy half 0
#      DVE: recip (1024)          | stt psum+x half 1
#      GPS: attn-mul (1024, one op)
import ml_dtypes  # noqa: F401
import numpy as np

B, C, H, W = 8, 128, 128, 128
N = H * W
PP = 8
NKEYS = 64
PIECE = 2048      # phase-0 dma piece = 16 rows of H = one pool-block row
NPIECE = N // PIECE
GRP = 512
PAIR = 2 * GRP    # phase-1 iteration width
NPAIR = N // PAIR
SKEW = 3          # phase-1 pipeline depth between energy(g) and out(g-SKEW)

_CACHE = {}


def _build_nc():
    import concourse.bass as bass  # noqa: F401
    from concourse import bacc, mybir
    import concourse.tile as tile

    f32 = mybir.dt.float32
    bf16 = mybir.dt.bfloat16
    AF = mybir.ActivationFunctionType
    MUL = mybir.AluOpType.mult
    ADD = mybir.AluOpType.add

    nc = bacc.Bacc(None, target_bir_lowering=False)

    x_d = nc.dram_tensor("x", [C, N], bf16, kind="ExternalInput")
    # bf16 weights: [WmT0 | WmT1 | WksT0 | WksT1 | idb | ones_bd]
    wb_d = nc.dram_tensor("wb", [C, 6 * C], bf16, kind="ExternalInput")
    # f32 weights: [bks0 bks1 bq gvec]
    wf_d = nc.dram_tensor("wf", [C, 7], f32, kind="ExternalInput")
    out_d = nc.dram_tensor("out", [C, N], bf16, kind="ExternalOutput")

    with tile.TileContext(nc) as tc:
        with (
            tc.tile_pool(name="const", bufs=1) as const,
            tc.tile_pool(name="big", bufs=1) as big,
            tc.tile_pool(name="tree", bufs=2) as tree,
            tc.tile_pool(name="expp", bufs=5) as expp,
            tc.tile_pool(name="rp", bufs=4) as rp,
            tc.tile_pool(name="attnp", bufs=5) as attnp,
            tc.tile_pool(name="outp", bufs=5) as outp,
        ):
            wb = const.tile([C, 6 * C], bf16)
            wf = const.tile([C, 7], f32)
            wmT0 = wb[:, 0:C]
            wmT1 = wb[:, C:2 * C]
            wksT0 = wb[:, 2 * C:3 * C]
            wksT1 = wb[:, 3 * C:4 * C]
            idb = wb[:, 4 * C:5 * C]
            ones_bd = wb[:, 5 * C:6 * C]
            bks0 = wf[:, 0:1]
            bks1 = wf[:, 1:2]
            gvec = wf[:, 3:4]
            wvb = wf[:, 4:6]   # Wks_g^T bq per branch (for the fast ebias)
            cvec = wf[:, 6:7]  # bq . bks_g per key's branch

            x_bf = big.tile([C, N], bf16)
            xp = big.tile([C, NKEYS], f32)
            xp_bf = big.tile([C, NKEYS], bf16)
            m_bf = big.tile([C, C], bf16)
            k_bf = big.tile([C, C], bf16)
            kT = big.tile([C, C], bf16)
            ebias = big.tile([C, 1], f32)
            tiny = big.tile([C, 1], f32)

            # ---- phase 0: stream x (bf16); pool on the PE ----
            # Row sums of each 16-row pool block accumulate in PSUM via 16
            # identity matmuls per piece (exact f32 adds, and the steady MM
            # stream holds the PE clock gate at 2.4 GHz through the mid with
            # no separate warm burn).  DVE only does a tiny 128-free reduce
            # over w=16 per piece.
            ph_pp = tc.tile_pool(name="ps_pp", bufs=1, space="PSUM")
            ps_pp = ph_pp.__enter__()
            pp_ps = ps_pp.tile([C, NPIECE * H // 2], f32, tag="poolps")
            scr_ps = ps_pp.tile([C, C], f32, tag="scr")

            # x head pieces first, then weights - input stream is critical
            for p in range(2):
                psl = bass.ts(p, PIECE)
                nc.sync.dma_start(x_bf[:, psl], x_d[:, psl])
            nc.sync.dma_start(wb[:], wb_d[:])
            nc.sync.dma_start(wf[:], wf_d[:])
            # preload the exp table set + prime small constants
            nc.scalar.activation(tiny[:], wf[:, 0:1], AF.Exp)
            wvb_bf = big.tile([C, 2], bf16)
            nc.vector.tensor_copy(wvb_bf[:], wvb)
            for p in range(NPIECE):
                base = p * PIECE
                if p == NPIECE - 1:
                    # split the final piece so its pooling can start on the
                    # first 8 rows while the last 8 stream in
                    nc.sync.dma_start(
                        x_bf[:, bass.ds(base, PIECE // 2)],
                        x_d[:, bass.ds(base, PIECE // 2)],
                    )
                    nc.sync.dma_start(
                        x_bf[:, bass.ds(base + PIECE // 2, PIECE // 2)],
                        x_d[:, bass.ds(base + PIECE // 2, PIECE // 2)],
                    )
                elif p >= 2:
                    nc.sync.dma_start(
                        x_bf[:, bass.ts(p, PIECE)], x_d[:, bass.ts(p, PIECE)]
                    )
                if p % 2 == 1:
                    # odd pieces (incl. the split tail): PE row-sums via 16
                    # accumulating identity matmuls into PSUM + tiny reduce
                    pps = pp_ps[:, (p // 2) * H:(p // 2 + 1) * H]
                    for r in range(16):
                        nc.tensor.matmul(
                            pps, idb, x_bf[:, bass.ds(base + r * H, H)],
                            start=(r == 0), stop=(r == 15),
                        )
                    ppv = pps.rearrange("p (pj w) -> p pj w", pj=PP, w=16)
                    nc.vector.tensor_reduce(
                        xp[:, p * PP:(p + 1) * PP], ppv,
                        axis=mybir.AxisListType.X, op=ADD,
                    )
                else:
                    # even pieces: DVE bf16 2x tree (rows 16->8->4) + reduce
                    t1 = tree.tile([C, PIECE // 2], bf16, name="t1")
                    t2 = tree.tile([C, PIECE // 4], bf16, name="t2")
                    nc.vector.tensor_tensor(
                        out=t1[:], in0=x_bf[:, bass.ds(base, 1024)],
                        in1=x_bf[:, bass.ds(base + 1024, 1024)], op=ADD,
                    )
                    nc.vector.tensor_tensor(
                        out=t2[:], in0=t1[:, 0:512], in1=t1[:, 512:1024], op=ADD,
                    )
                    t2v = t2[:].rearrange("p (h pj w) -> p pj h w", h=4, pj=PP, w=16)
                    nc.vector.tensor_reduce(
                        xp[:, p * PP:(p + 1) * PP], t2v,
                        axis=mybir.AxisListType.XY, op=ADD,
                    )
                if p == 4:
                    # contiguous ~4.8us matmul burn: the parity-split pool
                    # matmuls alone are too sparse to trip the HAM activity
                    # window, leaving the whole mid at 1.2 GHz
                    for _ in range(45):
                        nc.tensor.matmul(
                            scr_ps[:], ones_bd, x_bf[:, bass.ds(base, C)],
                            start=True, stop=True,
                        )

            ph_pp.__exit__(None, None, None)

            # ---- mid: M (energy weights), keys, kT, ebias ----
            # Ordering: energy(0)+exp(0) are emitted BEFORE the kk/kT chain
            # so the first pair's PE+ACT work never queues behind the
            # transpose (which waits on the k_bf activations).  ps_e opens
            # while ps0 (4 banks) is still live: 4+4 = 8 banks exactly.
            ph_e = tc.tile_pool(name="ps_e", bufs=2, space="PSUM")
            ps_e = ph_e.__enter__()
            ph0 = tc.tile_pool(name="ps0", bufs=1, space="PSUM")
            ps0 = ph0.__enter__()
            m_ps = ps0.tile([C, C], f32, tag="mps")
            kk_ps = ps0.tile([C, C], f32, tag="kkps")
            eb_ps = ps0.tile([C, 1], f32, tag="ebps")
            kT_ps = ps0.tile([C, C], bf16, tag="ktps")

            nc.vector.tensor_copy(xp_bf[:], xp[:])
            nc.tensor.matmul(m_ps[:, 0:NKEYS], wmT0, xp_bf[:], start=True, stop=True)
            nc.tensor.matmul(m_ps[:, NKEYS:], wmT1, xp_bf[:], start=True, stop=True)
            # fast ebias: ebias[key] = bq . k_key = (Wks^T bq) . xp_key + bq.bks
            # - straight from pooled x, off the k_bf critical path
            nc.tensor.matmul(
                eb_ps[0:NKEYS, :], xp_bf[:], wvb_bf[:, 0:1], start=True, stop=True
            )
            nc.tensor.matmul(
                eb_ps[NKEYS:, :], xp_bf[:], wvb_bf[:, 1:2], start=True, stop=True
            )
            # NOTE: the M bias (Wq^T bks) is a per-column constant within each
            # branch's key group, so softmax cancels it - no bias needed here.
            nc.scalar.copy(m_bf[:], m_ps[:])
            nc.vector.tensor_scalar_add(out=ebias[:], in0=eb_ps[:], scalar1=cvec)

            exps = [None] * NPAIR
            attns = [None] * NPAIR

            def stage_a(ga):
                a0 = bass.ds(ga * PAIR, GRP)
                a1 = bass.ds(ga * PAIR + GRP, GRP)
                e_ps = ps_e.tile([C, PAIR], f32, tag="eps")
                nc.tensor.matmul(
                    e_ps[:, 0:GRP], m_bf[:], x_bf[:, a0], start=True, stop=True
                )
                nc.tensor.matmul(
                    e_ps[:, GRP:], m_bf[:], x_bf[:, a1], start=True, stop=True
                )
                exps[ga] = expp.tile([C, PAIR], bf16, name="exp_sb")
                nc.scalar.activation(
                    exps[ga][:], e_ps[:], AF.Exp, bias=ebias[:, 0:1], scale=1.0
                )

            stage_a(0)

            # k/kT chain - only needed by out(0) at iteration SKEW, so it
            # runs in the PE/ACT shadow of the first exp
            nc.tensor.matmul(kk_ps[:, 0:NKEYS], wksT0, xp_bf[:], start=True, stop=True)
            nc.tensor.matmul(kk_ps[:, NKEYS:], wksT1, xp_bf[:], start=True, stop=True)
            nc.scalar.activation(
                k_bf[:, 0:NKEYS], kk_ps[:, 0:NKEYS], AF.Identity, bias=bks0, scale=1.0
            )
            nc.scalar.activation(
                k_bf[:, NKEYS:], kk_ps[:, NKEYS:], AF.Identity, bias=bks1, scale=1.0
            )
            nc.tensor.transpose(kT_ps[:], k_bf[:], idb)
            nc.scalar.activation(kT[:], kT_ps[:], AF.Copy, scale=gvec)
            ph0.__exit__(None, None, None)

            # ---- phase 1: deep-skewed pipeline over 1024-column pairs ----
            ph_s = tc.tile_pool(name="ps_s", bufs=1, space="PSUM")
            ps_s = ph_s.__enter__()
            ph_u = tc.tile_pool(name="ps_u", bufs=1, space="PSUM")
            ps_u = ph_u.__enter__()

            for g in range(1, NPAIR + SKEW):
                ga, gb, gc = g, g - 1, g - SKEW
                # oldest stage first so no engine queue ever waits behind a
                # younger, not-yet-ready instruction (head-of-line blocking)
                if 0 <= gc < NPAIR:
                    c0 = bass.ds(gc * PAIR, GRP)
                    c1 = bass.ds(gc * PAIR + GRP, GRP)
                    u0 = ps_u.tile([C, GRP], f32, tag="uh0")
                    u1 = ps_u.tile([C, GRP], f32, tag="uh1")
                    nc.tensor.matmul(
                        u0[:], kT[:], attns[gc][:, 0:GRP], start=True, stop=False
                    )
                    nc.tensor.matmul(
                        u1[:], kT[:], attns[gc][:, GRP:], start=True, stop=True
                    )
                    nc.tensor.matmul(
                        u0[:], idb, x_bf[:, c0], start=False, stop=True
                    )
                    o_sb = outp.tile([C, PAIR], bf16)
                    nc.scalar.activation(o_sb[:, 0:GRP], u0[:], AF.Copy)
                    nc.vector.scalar_tensor_tensor(
                        out=o_sb[:, GRP:],
                        in0=u1[:],
                        scalar=1.0,
                        in1=x_bf[:, c1],
                        op0=MUL,
                        op1=ADD,
                    )
                    if gc == NPAIR - 1:
                        # final pair: half DMAs shorten the critical tail
                        nc.sync.dma_start(
                            out_d[:, bass.ds(gc * PAIR, GRP)], o_sb[:, 0:GRP]
                        )
                        nc.sync.dma_start(
                            out_d[:, bass.ds(gc * PAIR + GRP, GRP)], o_sb[:, GRP:]
                        )
                    else:
                        nc.sync.dma_start(
                            out_d[:, bass.ds(gc * PAIR, PAIR)], o_sb[:]
                        )
                if 0 <= gb < NPAIR:
                    s_ps = ps_s.tile([C, PAIR], f32, tag="sps")
                    nc.tensor.matmul(
                        s_ps[:, 0:GRP], ones_bd, exps[gb][:, 0:GRP],
                        start=True, stop=True,
                    )
                    nc.tensor.matmul(
                        s_ps[:, GRP:], ones_bd, exps[gb][:, GRP:],
                        start=True, stop=True,
                    )
                    r_sb = rp.tile([C, PAIR], f32)
                    nc.vector.reciprocal_approx_fast(
                        out=r_sb[:, 0:GRP], in_=s_ps[:, 0:GRP]
                    )
                    nc.vector.reciprocal_approx_fast(
                        out=r_sb[:, GRP:], in_=s_ps[:, GRP:]
                    )
                    # attn-mul as a single gpsimd op (amortizes the Q7
                    # launch); for the last two pairs split it in halves so
                    # the drain's out-matmuls start off the first half early
                    at = attnp.tile([C, PAIR], bf16, name="attn")
                    if gb >= NPAIR - 2:
                        nc.gpsimd.tensor_mul(
                            at[:, 0:GRP], exps[gb][:, 0:GRP], r_sb[:, 0:GRP]
                        )
                        nc.gpsimd.tensor_mul(
                            at[:, GRP:], exps[gb][:, GRP:], r_sb[:, GRP:]
                        )
                    else:
                        nc.gpsimd.tensor_mul(at[:], exps[gb][:], r_sb[:])
                    attns[gb] = at
                if ga < NPAIR:
                    stage_a(ga)
                else:
                    # drain keepalive: enough matmul activity per drain
                    # iteration that the PE clock gate never re-throttles
                    # (cold MMs double every remaining chain latency)
                    f_ps = ps_s.tile([C, PAIR], f32, tag="sps")
                    for fh in range(2):
                        for _ in range(3):
                            nc.tensor.matmul(
                                f_ps[:, fh * GRP:(fh + 1) * GRP], ones_bd,
                                x_bf[:, bass.ds(fh * GRP, GRP)],
                                start=True, stop=True,
                            )
            ph_u.__exit__(None, None, None)
            ph_s.__exit__(None, None, None)
            ph_e.__exit__(None, None, None)

    nc.compile()
    return nc


def _get_nc():
    if "nc" not in _CACHE:
        _CACHE["nc"] = _build_nc()
    return _CACHE["nc"]


def _make_in_maps(x, Wq, bq, Wk, bk, Wk1, bk1, gamma, gamma1, aphal, aphal1):
    a0 = float(np.asarray(aphal).reshape(-1)[0])
    a1 = float(np.asarray(aphal1).reshape(-1)[0])
    g0 = float(np.asarray(gamma).reshape(-1)[0])
    g1 = float(np.asarray(gamma1).reshape(-1)[0])

    f = np.float32
    Wq = np.asarray(Wq, f)
    Wks0 = np.asarray(Wk, f) * (a0 / 256.0)
    Wks1 = np.asarray(Wk1, f) * (a1 / 256.0)
    bks0 = np.asarray(bk, f).reshape(C) * a0
    bks1 = np.asarray(bk1, f).reshape(C) * a1
    wmT0 = Wks0.T @ Wq           # stationary for M = (Wq^T Wks) pool(x)
    wmT1 = Wks1.T @ Wq
    eye = np.eye(C, dtype=f)
    ones_bd = np.kron(np.eye(2, dtype=f), np.ones((NKEYS, NKEYS), f))
    wb = np.concatenate(
        [wmT0, wmT1, Wks0.T, Wks1.T, eye, ones_bd], axis=1
    ).astype("bfloat16")
    gvec = np.concatenate(
        [np.full((NKEYS, 1), g0 / a0, f), np.full((NKEYS, 1), g1 / a1, f)]
    )
    bqv = np.asarray(bq, f).reshape(C)
    wvb0 = Wks0.T @ bqv
    wvb1 = Wks1.T @ bqv
    cvec = np.concatenate(
        [np.full((NKEYS, 1), float(bqv @ bks0), f),
         np.full((NKEYS, 1), float(bqv @ bks1), f)]
    )
    wf = np.concatenate(
        [
            bks0.reshape(C, 1), bks1.reshape(C, 1),
            bqv.reshape(C, 1),
            gvec,
            wvb0.reshape(C, 1), wvb1.reshape(C, 1),
            cvec,
        ],
        axis=1,
    ).astype(f)
    wb = np.ascontiguousarray(wb)
    wf = np.ascontiguousarray(wf)
    in_maps = []
    for b in range(B):
        xb = np.ascontiguousarray(
            np.asarray(x)[b].reshape(C, N).astype("bfloat16")
        )
        in_maps.append({
            "x": xb,
            "wb": wb,
            "wf": wf,
        })
    return in_maps


def kernel(x, Wq, bq, Wk, bk, Wk1, bk1, gamma, gamma1, aphal, aphal1, **_):
    from concourse.bass_utils import run_bass_kernel_spmd

    nc = _get_nc()
    in_maps = _make_in_maps(
        np.asarray(x), np.asarray(Wq), np.asarray(bq), np.asarray(Wk),
        np.asarray(bk), np.asarray(Wk1), np.asarray(bk1), np.asarray(gamma),
        np.asarray(gamma1), np.asarray(aphal), np.asarray(aphal1),
    )
    res = None
    last_exc = None
    for _attempt in range(3):
        try:
            res = run_bass_kernel_spmd(nc, in_maps, core_ids=list(range(B)))
            break
        except Exception as e:  # transient NRT_EXEC_UNIT_UNRECOVERABLE faults
            last_exc = e
            import time as _time
            _time.sleep(2.0)
    if res is None:
        raise last_exc
    out = np.stack([
        res.results[b]["out"].astype(np.float32).reshape(C, H, W)
        for b in range(B)
    ])
    return out
